# revision 44
# baseline (speedup 1.0000x reference)
"""BinaryConv2D Trainium2 kernel — 1D Winograd F(2,3) along image width.

Full computation:
  out = conv2d(sign(pad(x)), sign(k)) * avgpool3x3(mean|pad(x)|_ci) * alpha + bias

The 3x3 conv is computed as a vertical-direct x horizontal-Winograd hybrid:
per output column pair (2c, 2c+1), F(2,3) gives 4 products m1..m4 from the
transformed inputs V1..V4 (values in {0,+-2}, exact in fp8) and transformed
weights U1..U4 (values in {+-0.5,+-1.5,+-1}, exact in fp8):

  y_even = m1 + m2 + m3        y_odd = m2 - m3 - m4

The vertical 3 taps stay direct: each m_i accumulates 3 row-shifted matmuls
in PSUM (fp8 DoubleRow, 256-wide contraction).  To minimize both PE streams
and DVE combine work, the comps are split over 3 PSUM banks:

  B1 = m2 (3 MMs)   B2 = m1 + m3 (6 MMs)   B3 = -m3 - m4 (6 MMs)
  y_even = B1 + B2             y_odd = B1 + B3

15 tap-instance streams per column pair replace direct conv's 18 -> the
tensor-engine stream drops from ~105us to ~78us per core.  Epilogue per
group: ScalarE drains B1 to SBUF; DVE does two fused scalar_tensor_tensor
ops per parity ((B add sb), then (t*alpha)*K); ScalarE adds bias.
Everything is exact integer/quarter arithmetic until the K*alpha scaling.

Device strategy: 8 NeuronCores, data-parallel over batch N=32 -> 4 img/core.
"""

import sys

for _p in ("/root/.axon_site/_ro/trn_rl_repo", "/opt/trn_rl_repo"):
    if _p not in sys.path:
        sys.path.append(_p)

import numpy as np
import ml_dtypes

import concourse.bass as bass  # noqa: F401  (registers arch tables)
import concourse.mybir as mybir
import concourse.tile as tile
from concourse import bacc
from concourse.bass_utils import run_bass_kernel_spmd

FP8 = mybir.dt.float8e4
F32 = mybir.dt.float32
BF16 = mybir.dt.bfloat16

NCORES = 8
N, H, W, C = 32, 56, 56, 256
HP, WP = H + 2, W + 2           # padded spatial 58x58
NIMG = N // NCORES              # images per core
TC = W // 2                     # 28 tile columns (output col pairs)
NINST = 15                      # weight instances (15 tap-streams)
OPIX = H * W                    # 3136 outputs per (img, chunk) in packed order

# output row groups: 16,16,16,8 rows; FD = rows*28 <= 448 (PSUM bank 512 f32)
GROUP_ROWS = (16, 16, 16, 8)
GROUP_ROW0 = (0, 16, 32, 48)
FDMAX = 16 * TC                 # 448

# V planes split into 4 per-group row-pieces per image so each piece's DMA
# is hidden behind the previous group's matmuls even on the first image.
# piece g covers padded rows GROUP_ROW0[g] .. GROUP_ROW0[g]+GROUP_ROWS[g]+1.
PIECE_ROWS = (18, 18, 18, 10)
PIECE_LEN = tuple(r * TC for r in PIECE_ROWS)   # 504,504,504,280
# no free-dim padding: only the ko-plane step (4*LEN) must be 16-aligned,
# and all 4*LEN here are.  Unpadded keeps the HBM image of each partition
# fully contiguous -> few large DMA descriptors.

# instance tables: which V plane each weight instance streams, which PSUM
# bank it accumulates into (see module docstring), and its vertical tap.
INST_COMP = [1, 1, 1, 0, 0, 0, 2, 2, 2, 2, 2, 2, 3, 3, 3]
INST_BANK = [0, 0, 0, 1, 1, 1, 1, 1, 1, 2, 2, 2, 2, 2, 2]
INST_TAP = [0, 1, 2, 0, 1, 2, 0, 1, 2, 0, 1, 2, 0, 1, 2]

_NC = None


def _build_nc():
    nc = bacc.Bacc("TRN2", target_bir_lowering=False, debug=False)

    xg = [
        nc.dram_tensor(f"xg{g}", [NIMG, 128, 2, 4, PIECE_LEN[g]], FP8,
                       kind="ExternalInput")
        for g in range(4)
    ]
    wb = nc.dram_tensor("wb", [128, NINST, 2, C], FP8, kind="ExternalInput")
    # K pre-replicated across partitions on host (bf16): avoids GpSimd
    # PartitionBroadcast, whose library-reload vs Multiply stalls the chip
    kb = nc.dram_tensor("kb", [NIMG, 128, OPIX], BF16, kind="ExternalInput")
    ab = nc.dram_tensor("ab", [128, 2], F32, kind="ExternalInput")
    bb = nc.dram_tensor("bb", [128, 2], F32, kind="ExternalInput")
    ob = nc.dram_tensor("ob", [NIMG, 2, 128, OPIX], BF16, kind="ExternalOutput")

    with tile.TileContext(nc) as tc:
        with (
            tc.tile_pool(name="wp", bufs=1) as wp,
            tc.tile_pool(name="xp", bufs=3) as xp,
            tc.tile_pool(name="kp", bufs=4) as kp,
            tc.tile_pool(name="ep", bufs=4) as ep,
            tc.tile_pool(name="op", bufs=6) as op,
            tc.tile_pool(name="ps", bufs=6, space="PSUM") as ps,
        ):
            def dma_img(img):
                # first-use order: piece0, piece1, K (group-0 epilogue),
                # piece2, piece3.  K must not come later: engine queues are
                # strict FIFO, so a K-blocked multiply at the Vector queue
                # head would stall every DVE op behind it.
                pieces = [None] * 4
                khalf = GROUP_ROW0[2] * W  # K for groups 0-1 / 2-3
                for g in (0, 1):
                    pieces[g] = xp.tile(
                        [128, 2, 4, PIECE_LEN[g]], FP8,
                        name=f"xg{g}", tag=f"xg{g}",
                    )
                    nc.sync.dma_start(pieces[g][:], xg[g][img])
                k_sb = kp.tile([128, OPIX], BF16, tag="kbig")
                nc.sync.dma_start(k_sb[:, :khalf], kb[img, :, :khalf])
                for g in (2, 3):
                    pieces[g] = xp.tile(
                        [128, 2, 4, PIECE_LEN[g]], FP8,
                        name=f"xg{g}", tag=f"xg{g}",
                    )
                    nc.sync.dma_start(pieces[g][:], xg[g][img])
                nc.sync.dma_start(k_sb[:, khalf:], kb[img, :, khalf:])
                return pieces, k_sb

            w_sb = wp.tile([128, NINST, 2, C], FP8)
            nc.sync.dma_start(w_sb[:], wb[:])
            a_sb = wp.tile([128, 2], F32, tag="a")
            nc.sync.dma_start(a_sb[:], ab[:])
            b_sb = wp.tile([128, 2], F32, tag="b")
            nc.sync.dma_start(b_sb[:], bb[:])

            # warm the PE clock (HAM) with matmuls on a memset scratch tile
            # while the first image's V planes are still in flight
            scr = wp.tile([128, 2, FDMAX], FP8, tag="scr")
            nc.vector.memset(scr[:], 0)
            warm_ps = ps.tile([128, FDMAX], F32, tag="pt", bufs=1)
            for _ in range(18):
                nc.tensor.matmul(
                    warm_ps[:],
                    scr[:, :, 0:128],
                    scr[:],
                    start=True,
                    stop=True,
                    perf_mode=mybir.MatmulPerfMode.DoubleRow,
                )

            for img in range(NIMG):
                pieces, k_sb = dma_img(img)

                for c in range(2):
                    for g in range(4):
                        rows = GROUP_ROWS[g]
                        fd = rows * TC
                        src = pieces[g]

                        banks = [
                            ps.tile(
                                [128, FDMAX],
                                F32,
                                name=f"bank{j}",
                                tag=f"bank{j}",
                                bufs=3 if j == 0 else 2,
                            )
                            for j in range(3)
                        ]
                        started = [False, False, False]
                        for i in range(NINST):
                            bk = INST_BANK[i]
                            off = INST_TAP[i] * TC
                            last = (
                                i == 2 if bk == 0
                                else i == 8 if bk == 1
                                else i == 14
                            )
                            nc.tensor.matmul(
                                banks[bk][:, :fd],
                                w_sb[:, i, :, c * 128 : (c + 1) * 128],
                                src[:, :, INST_COMP[i], off : off + fd],
                                start=not started[bk],
                                stop=last,
                                perf_mode=mybir.MatmulPerfMode.DoubleRow,
                            )
                            started[bk] = True

                        # epilogue: y_even = B1+B2, y_odd = B1+B3, then
                        # *K*alpha (fused) and +bias
                        goff = GROUP_ROW0[g] * W
                        sb = ep.tile([128, FDMAX], F32, tag="sb")
                        nc.scalar.copy(sb[:, :fd], banks[0][:, :fd])
                        o_sb = op.tile([128, 2 * FDMAX], BF16, tag="o")
                        for par, bank in ((0, banks[1]), (1, banks[2])):
                            t = ep.tile([128, FDMAX], F32, tag=f"t{par}")
                            nc.vector.scalar_tensor_tensor(
                                t[:, :fd],
                                bank[:, :fd],
                                0.0,
                                sb[:, :fd],
                                mybir.AluOpType.bypass,
                                mybir.AluOpType.add,
                            )
                            y = ep.tile([128, FDMAX], F32, tag=f"y{par}")
                            ksl = k_sb[:, goff + par * fd : goff + (par + 1) * fd]
                            # K-multiply: even parity on DVE, odd on GpSimd to
                            # keep every engine under the PE stream time; the
                            # final image's last chunk goes all-DVE so the
                            # drain tail isn't gated by the slow GpSimd op
                            tail_unit = img == NIMG - 1 and g == 3
                            if par == 0 or tail_unit:
                                nc.vector.tensor_tensor(
                                    y[:, :fd], t[:, :fd], ksl, mybir.AluOpType.mult
                                )
                            else:
                                nc.gpsimd.tensor_tensor(
                                    y[:, :fd], t[:, :fd], ksl, mybir.AluOpType.mult
                                )
                            # alpha folded into the ScalarE affine: y*alpha+beta
                            nc.scalar.activation(
                                o_sb[:, par * fd : (par + 1) * fd],
                                y[:, :fd],
                                mybir.ActivationFunctionType.Identity,
                                bias=b_sb[:, c : c + 1],
                                scale=a_sb[:, c : c + 1],
                            )
                        nc.sync.dma_start(
                            ob[img, c, :, goff : goff + 2 * fd],
                            o_sb[:, : 2 * fd],
                        )

    nc.compile()
    return nc


def get_nc():
    global _NC
    if _NC is None:
        _NC = _build_nc()
    return _NC


def prep_inputs(x, kernel, bias):
    """Host-side prep: binarize, pad, Winograd-transform; per-core in_maps."""
    np_fp8 = mybir.dt.np(FP8)
    xp = np.pad(x, ((0, 0), (1, 1), (1, 1), (0, 0)))
    binx = np.where(xp > 0, np.float32(1.0), np.float32(-1.0))
    b = np.ascontiguousarray(binx.transpose(0, 3, 1, 2))  # (N, 256, 58, 58)
    d0 = b[..., 0:56:2]
    d1 = b[..., 1:57:2]
    d2 = b[..., 2:58:2]
    d3 = b[..., 3::2]
    # V planes (N, 256, 4, 58, 28), values in {0, +-2}: exact in fp8
    V = np.stack([d0 - d2, d1 + d2, d2 - d1, d1 - d3], axis=2).astype(np_fp8)
    V = V.reshape(N, 2, 128, 4, HP * TC)  # ci = ci_hi*128 + ci_lo

    # [N, 128part, 2ko, 4comp, len]: per-partition HBM image contiguous
    xg_all = []
    for g in range(4):
        o0 = GROUP_ROW0[g] * TC
        xg_all.append(np.ascontiguousarray(
            V[:, :, :, :, o0 : o0 + PIECE_LEN[g]].transpose(0, 2, 1, 3, 4)
        ))

    # K = avgpool3x3(beta), packed per group as [g, parity, row, tilecol],
    # replicated to all 128 partitions, bf16
    beta = np.abs(xp).mean(axis=3)
    ks = beta[:, 0:H, :] + beta[:, 1 : H + 1, :] + beta[:, 2 : H + 2, :]
    K = (ks[:, :, 0:W] + ks[:, :, 1 : W + 1] + ks[:, :, 2 : W + 2]) / np.float32(9.0)
    K_flat = np.empty((N, OPIX), dtype=ml_dtypes.bfloat16)
    for g in range(4):
        r0, rows = GROUP_ROW0[g], GROUP_ROWS[g]
        seg = K[:, r0 : r0 + rows, :].reshape(N, rows, TC, 2)  # [r, c, par]
        K_flat[:, r0 * W : (r0 + rows) * W] = seg.transpose(0, 3, 1, 2).reshape(
            N, rows * W
        )
    K_pack = np.ascontiguousarray(
        np.broadcast_to(K_flat[:, None, :], (N, 128, OPIX))
    )

    # Winograd weight instances (15): see module docstring
    g3 = np.where(kernel > 0, np.float32(1.0), np.float32(-1.0))  # (3,3,256,256)
    U1 = g3[:, 0]
    U2 = (g3[:, 0] + g3[:, 1] + g3[:, 2]) / np.float32(2.0)
    U3 = (g3[:, 0] - g3[:, 1] + g3[:, 2]) / np.float32(2.0)
    U4 = g3[:, 2]
    # instance list: (U comp, sign) aligned with INST_* tables
    inst = [U2, U2, U2, U1, U1, U1, U3, U3, U3, -U3, -U3, -U3, -U4, -U4, -U4]
    wb = np.empty((128, NINST, 2, C), dtype=np_fp8)
    for i in range(NINST):
        u = inst[i][INST_TAP[i]].reshape(2, 128, C)  # (ci_hi, ci_lo, co)
        wb[:, i] = u.transpose(1, 0, 2).astype(np_fp8)

    alpha = np.abs(kernel).mean(axis=(0, 1, 2)).astype(np.float32)
    ab = np.ascontiguousarray(alpha.reshape(2, 128).T)
    bb = np.ascontiguousarray(bias.astype(np.float32).reshape(2, 128).T)

    in_maps = []
    for core in range(NCORES):
        sl = slice(core * NIMG, (core + 1) * NIMG)
        im = {
            "kb": np.ascontiguousarray(K_pack[sl]),
            "wb": wb,
            "ab": ab,
            "bb": bb,
        }
        for g in range(4):
            im[f"xg{g}"] = np.ascontiguousarray(xg_all[g][sl])
        in_maps.append(im)
    return in_maps


def assemble_output(results):
    """(8 cores x (NIMG, 2, 128, OPIX)) -> (N, H, W, C) f32."""
    ot = np.concatenate([r["ob"] for r in results], axis=0).astype(
        np.float32
    )  # (N, 2, 128, OPIX)
    out = np.empty((N, H, W, C), dtype=np.float32)
    for g in range(4):
        r0, rows = GROUP_ROW0[g], GROUP_ROWS[g]
        seg = ot[:, :, :, r0 * W : (r0 + rows) * W].reshape(
            N, 2, 128, 2, rows, TC
        )  # [n, chunk, colo, par, r, c]
        out[:, r0 : r0 + rows] = (
            seg.transpose(0, 4, 5, 3, 1, 2).reshape(N, rows, W, C)
        )
    return out


def kernel(x, kernel, bias, _trace=False):
    nc = get_nc()
    in_maps = prep_inputs(x, kernel, bias)
    res = run_bass_kernel_spmd(
        nc, in_maps, core_ids=list(range(NCORES)), trace=_trace
    )
    out = assemble_output(res.results)
    if _trace:
        return out, res
    return out
